# revision 24
# baseline (speedup 1.0000x reference)
"""Trainium2 Bass kernel for a Bahdanau-style attention module.

Reference computation (B=64, S=512, H=1000, D=2H=2000):
    ci   = context @ W_a.T                        # (B,S,H)
    hi   = decoder_hidden @ W_b.T                 # (1,B,H) -> (B,1,H)
    att  = tanh(ci + hi) @ W_c_w.T + W_c_b        # (B,S,1)
    att  = where(mask, -1e6, att); att = softmax(att, axis=1)
    ws   = att.T @ context                        # (B,1,2H)
    out  = ws @ dense_w.T + dense_b               # (B,1,H)

Strategy: data-parallel over batch across 8 NeuronCores (8 batches per
core, weights replicated; no collectives).  Inputs are pre-padded
(H->1024, 2H->2048), pre-cast to bf16 and packed partition-major on the
host so every DMA is a contiguous 128-partition load and every matmul
contraction dim lands on partitions.
"""

import numpy as np
import ml_dtypes

import concourse.bass as bass
import concourse.tile as tile
from concourse import bacc, mybir
from concourse.bass_utils import run_bass_kernel_spmd
from concourse.masks import make_identity

BF16 = ml_dtypes.bfloat16
FP8 = ml_dtypes.float8_e4m3
WA_SCALE = 64.0

B = 64          # global batch
BC = 8          # batches per core
NCORES = 8
S = 512         # source length
H = 1000
HP = 1024       # padded hidden
D = 2000
DP = 2048       # padded 2*hidden
KD = DP // 128  # 16 k-tiles over padded contraction dim
KH = HP // 128  # 8 h-tiles
KS = S // 128   # 4 s-tiles
F32 = mybir.dt.float32
BF = mybir.dt.bfloat16
F8 = mybir.dt.float8e4


def _pack_ktiles(a2d):
    """(K*128, N) -> (128, K*N) with [p, k*N+n] = a[k*128+p, n]."""
    k128, n = a2d.shape
    k = k128 // 128
    return np.ascontiguousarray(
        a2d.reshape(k, 128, n).transpose(1, 0, 2).reshape(128, k * n)
    )


def _build_graph():
    nc = bacc.Bacc()

    ctxT = nc.declare_dram_parameter("ctxT", [BC, 128, KD, S], F8, isOutput=False)
    ctxN = nc.declare_dram_parameter("ctxN", [BC, 128, KS * DP], BF, isOutput=False)
    waT = nc.declare_dram_parameter("waT", [128, KD, HP], F8, isOutput=False)
    wbT = nc.declare_dram_parameter("wbT", [128, KH * HP], BF, isOutput=False)
    dwT = nc.declare_dram_parameter("dwT", [128, KD * HP], BF, isOutput=False)
    hT = nc.declare_dram_parameter("hT", [128, KH * BC], BF, isOutput=False)
    wcT = nc.declare_dram_parameter("wcT", [128, KH], BF, isOutput=False)
    maskv = nc.declare_dram_parameter("maskv", [1, BC * S], F32, isOutput=False)
    dbias = nc.declare_dram_parameter("dbias", [128, 512], F32, isOutput=False)
    out_ext = nc.declare_dram_parameter("out", [2, BC, 512], F32, isOutput=True)

    with tile.TileContext(nc) as tc:
        with (
            tc.tile_pool(name="const", bufs=1) as cpool,
            tc.tile_pool(name="ctxTp", bufs=2) as ctxT_pool,
            tc.tile_pool(name="ctxNp", bufs=2) as ctxN_pool,
            tc.tile_pool(name="tanhp", bufs=3) as tanh_pool,
            tc.tile_pool(name="oncep", bufs=1) as once_pool,
            tc.tile_pool(name="smallp", bufs=2) as small_pool,
            tc.tile_pool(name="ci", bufs=3, space="PSUM") as ci_pool,
            tc.tile_pool(name="scps", bufs=3, space="PSUM") as sc_pool,
            tc.tile_pool(name="wsacc", bufs=2, space="PSUM") as wsacc_pool,
        ):
            # ---- resident weights / constants -------------------------------
            # DMA order matters for startup latency: the hid phase needs
            # hT+wbT, the first big matmuls need the first chunk of waT8;
            # dwT/dbias are tail-only and issued mid-loop.
            waT_sb = cpool.tile([128, KD, HP], F8, tag="waT")
            wbT_sb = cpool.tile([128, KH * HP], BF, tag="wbT")
            hT_sb = cpool.tile([128, KH * BC], BF, tag="hT")
            # startup-critical order: batch-0 context + W_a chunks first
            # (the first ci accumulation needs all of them), then the
            # hid-phase inputs, then everything else
            ctxT0_t = ctxT_pool.tile([128, KD, S], F8, tag="ctxT")
            for c in range(4):
                nc.sync.dma_start(
                    waT_sb[:, 4 * c : 4 * (c + 1), :],
                    waT[:, 4 * c : 4 * (c + 1), :],
                )
                nc.sync.dma_start(
                    ctxT0_t[:, 4 * c : 4 * (c + 1), :],
                    ctxT[0, :, 4 * c : 4 * (c + 1), :],
                )
                if c == 1:
                    with tc.high_priority():
                        nc.sync.dma_start(hT_sb[:], hT[:])
                        nc.sync.dma_start(wbT_sb[:], wbT[:])
            wcT_sb = cpool.tile([128, KH], BF, tag="wcT")
            nc.sync.dma_start(wcT_sb[:], wcT[:])
            maskv_sb = cpool.tile([1, BC * S], F32, tag="maskv")
            nc.sync.dma_start(maskv_sb[:], maskv[:])
            dwT_sb = cpool.tile([128, KD * HP], BF, tag="dwT")
            dbias_sb = cpool.tile([128, 512], F32, tag="dbias")

            # PE warmup: the first ~15us are DMA-bound and the PE would sit
            # idle and HAM-throttled; chew on zeros to enter the 2.4 GHz
            # state before the real matmuls arrive.
            warm_sb = cpool.tile([128, 512], BF, tag="warm")
            nc.gpsimd.memset(warm_sb[:], 0.0)
            warm_ps = wsacc_pool.tile([128, 512], F32, tag="wsacc", name="warmps")
            for _w in range(38):
                nc.tensor.matmul(
                    warm_ps[:],
                    warm_sb[:, 0:128],
                    warm_sb[:],
                    start=True,
                    stop=True,
                    skip_group_check=True,
                )
            warm_out = cpool.tile([1, 16], F32, tag="warmout")
            nc.vector.tensor_copy(warm_out[:], warm_ps[0:1, 0:16])

            ident_b = cpool.tile([128, 128], BF, tag="identb")
            make_identity(nc, ident_b[:])
            ident_f = cpool.tile([128, 128], F32, tag="identf")
            make_identity(nc, ident_f[:])

            # assembled per-batch results
            hidT_sb = cpool.tile([128, KH * BC], F32, tag="hidT")
            wsT_sb = cpool.tile([128, KD * BC], BF, tag="wsT")

            # ---- phase 0: hidden_in = decoder_hidden @ W_b.T ----------------
            # (emitted mid-way through batch 0's big matmuls so the PE does
            # not idle on the wbT DMA at startup)
            def hid_phase():
                hid_sb = once_pool.tile([128, 512], F32, tag="hid")
                psum_hid = wsacc_pool.tile([128, 512], F32, tag="wsacc")
                for k in range(KH):
                    for n in range(2):
                        nc.tensor.matmul(
                            psum_hid[32 * n : 32 * n + BC, :],
                            hT_sb[:, k * BC : (k + 1) * BC],
                            wbT_sb[:, k * HP + n * 512 : k * HP + (n + 1) * 512],
                            start=(k == 0),
                            stop=(k == KH - 1),
                            tile_position=(0, 32 * n),
                            skip_group_check=True,
                        )
                nc.vector.tensor_copy(hid_sb[:], psum_hid[:])
                for h in range(KH):
                    n, hh = divmod(h, 4)
                    pt = sc_pool.tile([128, BC], F32, tag="sc")
                    nc.tensor.transpose(
                        pt[:],
                        hid_sb[32 * n : 32 * n + BC, hh * 128 : (hh + 1) * 128],
                        ident_f[32 * n : 32 * n + BC, 32 * n : 32 * n + BC],
                        tile_position=(32 * n, 0),
                    )
                    nc.vector.tensor_copy(
                        hidT_sb[:, h * BC : (h + 1) * BC], pt[:]
                    )

            # ---- main pipeline over batches ---------------------------------
            ctxN_tiles = [None] * BC
            att_tiles = [None] * BC
            ws_psum = wsacc_pool.tile([128, 512], F32, tag="wsacc", name="wsps")

            def stage_scores(b):
                """big matmul + tanh + scores + masked softmax for batch b."""
                if b == 0:
                    ctxT_t = ctxT0_t
                else:
                    ctxT_t = ctxT_pool.tile([128, KD, S], F8, tag="ctxT")
                    nc.sync.dma_start(ctxT_t[:], ctxT[b])
                if b == 1:
                    # tail-only data; issued here so it doesn't delay startup
                    nc.sync.dma_start(dwT_sb[:], dwT[:])
                    nc.sync.dma_start(dbias_sb[:], dbias[:])
                ctxN_t = ctxN_pool.tile([128, KS * DP], BF, tag="ctxN")
                nc.sync.dma_start(ctxN_t[:], ctxN[b])
                ctxN_tiles[b] = ctxN_t

                psum_sc = sc_pool.tile([1, S], F32, tag="sc")
                tanh_tiles = {}
                ci_tiles = {}

                def emit_ci(h):
                    psum_ci = ci_pool.tile([128, S], F32, tag="ci")
                    for g in range(KD // 2):
                        nc.tensor.matmul(
                            psum_ci[:],
                            waT_sb[:, 2 * g : 2 * g + 2, h * 128 : (h + 1) * 128],
                            ctxT_t[:, 2 * g : 2 * g + 2, :],
                            start=(g == 0),
                            stop=(g == KD // 2 - 1),
                            perf_mode=mybir.MatmulPerfMode.DoubleRow,
                        )
                    ci_tiles[h] = psum_ci

                def emit_tanh(h):
                    tanh_t = tanh_pool.tile([128, S], BF, tag="tanh")
                    nc.scalar.activation(
                        tanh_t[:],
                        ci_tiles.pop(h)[:],
                        mybir.ActivationFunctionType.Tanh,
                        bias=hidT_sb[:, h * BC + b : h * BC + b + 1],
                        scale=1.0 / WA_SCALE,
                    )
                    tanh_tiles[h] = tanh_t

                def emit_scores(h):
                    nc.tensor.matmul(
                        psum_sc[:],
                        wcT_sb[:, h : h + 1],
                        tanh_tiles.pop(h)[:],
                        start=(h == 0),
                        stop=(h == KH - 1),
                    )

                # pipeline: tanh lags ci by `lag` h-blocks so the PE never
                # waits on ACT; scores lag tanh by one more.
                lag = 2 if b == 0 else 1
                for h in range(KH):
                    emit_ci(h)
                    if b == 0 and h == 2:
                        hid_phase()
                    if h >= lag:
                        emit_tanh(h - lag)
                    if h >= lag + 1:
                        emit_scores(h - lag - 1)
                for h in range(KH - lag, KH):
                    emit_tanh(h)
                for h in range(KH - lag - 1, KH):
                    emit_scores(h)

                # masked softmax on a single partition (512 elements)
                sc_sb = small_pool.tile([1, S], F32, tag="scsb")
                nc.vector.tensor_tensor(
                    sc_sb[:], psum_sc[:], maskv_sb[0:1, b * S : (b + 1) * S], op=mybir.AluOpType.add
                )
                # no max-subtraction: scores are O(1) (exp <= e^4) and
                # masked entries are -1e6 (exp underflows to exactly 0)
                exp_sb = small_pool.tile([1, S], F32, tag="exp")
                esum = small_pool.tile([1, 1], F32, tag="esum")
                nc.scalar.activation(
                    exp_sb[:], sc_sb[:], mybir.ActivationFunctionType.Exp,
                    bias=0.0, scale=1.0, accum_out=esum[:],
                )
                inv = small_pool.tile([1, 1], F32, tag="inv")
                nc.vector.reciprocal(inv[:], esum[:])
                att_sb = small_pool.tile([1, S], BF, tag="att")
                nc.vector.tensor_scalar_mul(att_sb[:], exp_sb[:], inv[:])
                att_tiles[b] = att_sb

            def stage_ws(b):
                """att transpose + weighted sum for batch b (accumulates into
                the persistent ws psum tiles; rows j != b add exactly zero
                because attT_b is zero outside column b)."""
                att_sb = att_tiles[b]
                attT_b = small_pool.tile([128, KS * BC], BF, tag="attTb")
                nc.gpsimd.memset(attT_b[:], 0.0)
                for st in range(KS):
                    pt = sc_pool.tile([128, 1], BF, tag="sc")
                    nc.tensor.transpose(
                        pt[:], att_sb[0:1, st * 128 : (st + 1) * 128],
                        ident_b[0:1, 0:1],
                    )
                    nc.vector.tensor_copy(
                        attT_b[:, st * BC + b : st * BC + b + 1], pt[:]
                    )
                ctxN_t = ctxN_tiles[b]
                for st in range(KS):
                    for nch in range(KS):
                        nc.tensor.matmul(
                            ws_psum[32 * nch : 32 * nch + BC, :],
                            attT_b[:, st * BC : (st + 1) * BC],
                            ctxN_t[:, st * DP + nch * 512 : st * DP + (nch + 1) * 512],
                            start=(b == 0 and st == 0),
                            stop=(b == BC - 1 and st == KS - 1),
                            tile_position=(0, 32 * nch),
                            skip_group_check=True,
                        )

            # software pipeline: scores(b) runs while ws(b-1) consumes
            for b in range(BC + 1):
                if b < BC:
                    stage_scores(b)
                if b >= 1:
                    stage_ws(b - 1)

            # ---- tail: dense layer ------------------------------------------
            ws_col = once_pool.tile([128, 512], BF, tag="wscol")
            nc.vector.tensor_copy(ws_col[:], ws_psum[:])
            for k in range(KD):
                nch, kk = divmod(k, KS)
                pt = sc_pool.tile([128, BC], BF, tag="sc")
                nc.tensor.transpose(
                    pt[:],
                    ws_col[32 * nch : 32 * nch + BC, kk * 128 : (kk + 1) * 128],
                    ident_b[32 * nch : 32 * nch + BC, 32 * nch : 32 * nch + BC],
                    tile_position=(32 * nch, 0),
                )
                nc.vector.tensor_copy(wsT_sb[:, k * BC : (k + 1) * BC], pt[:])

            psum_d = wsacc_pool.tile([128, 512], F32, tag="wsacc")
            for k in range(KD):
                for n in range(2):
                    nc.tensor.matmul(
                        psum_d[32 * n : 32 * n + BC, :],
                        wsT_sb[:, k * BC : (k + 1) * BC],
                        dwT_sb[:, k * HP + n * 512 : k * HP + (n + 1) * 512],
                        start=(k == 0),
                        stop=(k == KD - 1),
                        tile_position=(0, 32 * n),
                        skip_group_check=True,
                    )
            out_sb = once_pool.tile([128, 512], F32, tag="outsb")
            nc.vector.tensor_tensor(
                out_sb[:], psum_d[:], dbias_sb[:], op=mybir.AluOpType.add
            )
            for n in range(2):
                nc.sync.dma_start(out_ext[n], out_sb[32 * n : 32 * n + BC, :])

    nc.compile()
    return nc


_GRAPH = None


def _prep_inputs(decoder_hidden, context, mask, W_a, W_b, W_c_w, W_c_b,
                 dense_w, dense_b):
    """Shard + pad + cast + pack all inputs into per-core input maps."""
    # weights (replicated, packed partition-major over the contraction dim)
    wa = np.zeros((DP, HP), dtype=FP8)
    wa[:D, :H] = (W_a.T.astype(np.float32) * WA_SCALE).astype(FP8)
    waT_p = np.ascontiguousarray(wa.reshape(KD, 128, HP).transpose(1, 0, 2))
    wb = np.zeros((HP, HP), dtype=BF16)
    wb[:H, :H] = W_b.T.astype(BF16)
    wbT_p = _pack_ktiles(wb)
    dw = np.zeros((DP, HP), dtype=BF16)
    dw[:D, :H] = dense_w.T.astype(BF16)
    dwT_p = _pack_ktiles(dw)
    wc = np.zeros((HP, 1), dtype=BF16)
    wc[:H, 0] = W_c_w[0].astype(BF16)
    wcT_p = _pack_ktiles(wc)
    db = np.zeros((HP,), dtype=np.float32)
    db[:H] = dense_b.astype(np.float32)
    dbias_p = np.zeros((128, 512), dtype=np.float32)
    for n in range(2):
        dbias_p[32 * n : 32 * n + BC, :] = db[n * 512 : (n + 1) * 512]

    hid = np.zeros((HP, B), dtype=BF16)
    hid[:H, :] = decoder_hidden[0].T.astype(BF16)   # (H, B)

    maskf = W_c_b.astype(np.float32)[0] - 1e6 * mask[:, :, 0].astype(np.float32)

    in_maps = []
    for c in range(NCORES):
        b0 = c * BC
        ctxf = np.zeros((BC, S, DP), dtype=np.float32)
        ctxf[:, :, :D] = context[b0 : b0 + BC]
        # d-major fp8 packing: [b, p, k, s] = ctx[b, s, k*128+p]
        ctxT_p = np.ascontiguousarray(
            ctxf.transpose(0, 2, 1).astype(FP8).reshape(BC, KD, 128, S)
            .transpose(0, 2, 1, 3)
        )
        # s-major bf16 packing: [b, p, st*DP+d] = ctx[b, st*128+p, d]
        ctxN_p = np.ascontiguousarray(
            ctxf.astype(BF16).reshape(BC, KS, 128, DP).transpose(0, 2, 1, 3)
            .reshape(BC, 128, KS * DP)
        )
        hT_p = _pack_ktiles(np.ascontiguousarray(hid[:, b0 : b0 + BC]))
        in_maps.append({
            "ctxT": ctxT_p,
            "ctxN": ctxN_p,
            "waT": waT_p,
            "wbT": wbT_p,
            "dwT": dwT_p,
            "hT": hT_p,
            "wcT": wcT_p,
            "maskv": np.ascontiguousarray(maskf[b0 : b0 + BC].reshape(1, BC * S)),
            "dbias": dbias_p,
        })
    return in_maps


def kernel(decoder_hidden, context, mask, W_a, W_b, W_c_w, W_c_b,
           dense_w, dense_b, _trace=False):
    global _GRAPH
    if _GRAPH is None:
        _GRAPH = _build_graph()
    in_maps = _prep_inputs(
        np.asarray(decoder_hidden), np.asarray(context), np.asarray(mask),
        np.asarray(W_a), np.asarray(W_b), np.asarray(W_c_w),
        np.asarray(W_c_b), np.asarray(dense_w), np.asarray(dense_b),
    )
    res = run_bass_kernel_spmd(_GRAPH, in_maps, list(range(NCORES)), trace=_trace)
    out = np.concatenate(
        [np.concatenate([res.results[c]["out"][0], res.results[c]["out"][1]],
                        axis=1)[:, :H]
         for c in range(NCORES)], axis=0
    ).astype(np.float32)
    if _trace:
        kernel.last_exec_time_ns = res.exec_time_ns
    return out.reshape(B, 1, H)


# revision 25
# speedup vs baseline: 1.0505x; 1.0505x over previous
"""Trainium2 Bass kernel for a Bahdanau-style attention module.

Reference computation (B=64, S=512, H=1000, D=2H=2000):
    ci   = context @ W_a.T                        # (B,S,H)
    hi   = decoder_hidden @ W_b.T                 # (1,B,H) -> (B,1,H)
    att  = tanh(ci + hi) @ W_c_w.T + W_c_b        # (B,S,1)
    att  = where(mask, -1e6, att); att = softmax(att, axis=1)
    ws   = att.T @ context                        # (B,1,2H)
    out  = ws @ dense_w.T + dense_b               # (B,1,H)

Strategy: data-parallel over batch across 8 NeuronCores (8 batches per
core, weights replicated; no collectives).  Inputs are pre-padded
(H->1024, 2H->2048), pre-cast to bf16 and packed partition-major on the
host so every DMA is a contiguous 128-partition load and every matmul
contraction dim lands on partitions.
"""

import numpy as np
import ml_dtypes

import concourse.bass as bass
import concourse.tile as tile
from concourse import bacc, mybir
from concourse.bass_utils import run_bass_kernel_spmd
from concourse.masks import make_identity

BF16 = ml_dtypes.bfloat16
FP8 = ml_dtypes.float8_e4m3
WA_SCALE = 64.0

B = 64          # global batch
BC = 8          # batches per core
NCORES = 8
S = 512         # source length
H = 1000
HP = 1024       # padded hidden
D = 2000
DP = 2048       # padded 2*hidden
KD = DP // 128  # 16 k-tiles over padded contraction dim
KH = HP // 128  # 8 h-tiles
KS = S // 128   # 4 s-tiles
F32 = mybir.dt.float32
BF = mybir.dt.bfloat16
F8 = mybir.dt.float8e4


def _pack_ktiles(a2d):
    """(K*128, N) -> (128, K*N) with [p, k*N+n] = a[k*128+p, n]."""
    k128, n = a2d.shape
    k = k128 // 128
    return np.ascontiguousarray(
        a2d.reshape(k, 128, n).transpose(1, 0, 2).reshape(128, k * n)
    )


def _build_graph():
    nc = bacc.Bacc()

    ctxT = nc.declare_dram_parameter("ctxT", [BC, 128, KD, S], F8, isOutput=False)
    ctxN = nc.declare_dram_parameter("ctxN", [BC, 128, KS * DP], BF, isOutput=False)
    waT = nc.declare_dram_parameter("waT", [128, KD, HP], F8, isOutput=False)
    wbT = nc.declare_dram_parameter("wbT", [128, KH * HP], BF, isOutput=False)
    dwT = nc.declare_dram_parameter("dwT", [128, KD * HP], BF, isOutput=False)
    hT = nc.declare_dram_parameter("hT", [128, KH * BC], BF, isOutput=False)
    wcT = nc.declare_dram_parameter("wcT", [128, KH], BF, isOutput=False)
    maskv = nc.declare_dram_parameter("maskv", [1, BC * S], F32, isOutput=False)
    dbias = nc.declare_dram_parameter("dbias", [128, 512], F32, isOutput=False)
    out_ext = nc.declare_dram_parameter("out", [2, BC, 512], F32, isOutput=True)

    with tile.TileContext(nc) as tc:
        with (
            tc.tile_pool(name="const", bufs=1) as cpool,
            tc.tile_pool(name="ctxTp", bufs=2) as ctxT_pool,
            tc.tile_pool(name="ctxNp", bufs=2) as ctxN_pool,
            tc.tile_pool(name="tanhp", bufs=9) as tanh_pool,
            tc.tile_pool(name="oncep", bufs=1) as once_pool,
            tc.tile_pool(name="smallp", bufs=2) as small_pool,
            tc.tile_pool(name="ci", bufs=3, space="PSUM") as ci_pool,
            tc.tile_pool(name="scps", bufs=3, space="PSUM") as sc_pool,
            tc.tile_pool(name="wsacc", bufs=2, space="PSUM") as wsacc_pool,
        ):
            # ---- resident weights / constants -------------------------------
            # DMA order matters for startup latency: the hid phase needs
            # hT+wbT, the first big matmuls need the first chunk of waT8;
            # dwT/dbias are tail-only and issued mid-loop.
            waT_sb = cpool.tile([128, KD, HP], F8, tag="waT")
            wbT_sb = cpool.tile([128, KH * HP], BF, tag="wbT")
            hT_sb = cpool.tile([128, KH * BC], BF, tag="hT")
            # startup-critical order: batch-0 context + W_a chunks first
            # (the first ci accumulation needs all of them), then the
            # hid-phase inputs, then everything else
            ctxT0_t = ctxT_pool.tile([128, KD, S], F8, tag="ctxT")
            for c in range(4):
                nc.sync.dma_start(
                    waT_sb[:, 4 * c : 4 * (c + 1), :],
                    waT[:, 4 * c : 4 * (c + 1), :],
                )
                nc.sync.dma_start(
                    ctxT0_t[:, 4 * c : 4 * (c + 1), :],
                    ctxT[0, :, 4 * c : 4 * (c + 1), :],
                )
                if c == 1:
                    with tc.high_priority():
                        nc.sync.dma_start(hT_sb[:], hT[:])
                        nc.sync.dma_start(wbT_sb[:], wbT[:])
            wcT_sb = cpool.tile([128, KH], BF, tag="wcT")
            nc.sync.dma_start(wcT_sb[:], wcT[:])
            maskv_sb = cpool.tile([1, BC * S], F32, tag="maskv")
            nc.sync.dma_start(maskv_sb[:], maskv[:])
            dwT_sb = cpool.tile([128, KD * HP], BF, tag="dwT")
            dbias_sb = cpool.tile([128, 512], F32, tag="dbias")

            # PE warmup: the first ~15us are DMA-bound and the PE would sit
            # idle and HAM-throttled; chew on zeros to enter the 2.4 GHz
            # state before the real matmuls arrive.
            warm_sb = cpool.tile([128, 512], BF, tag="warm")
            nc.gpsimd.memset(warm_sb[:], 0.0)
            warm_ps = wsacc_pool.tile([128, 512], F32, tag="wsacc", name="warmps")
            for _w in range(38):
                nc.tensor.matmul(
                    warm_ps[:],
                    warm_sb[:, 0:128],
                    warm_sb[:],
                    start=True,
                    stop=True,
                    skip_group_check=True,
                )
            warm_out = cpool.tile([1, 16], F32, tag="warmout")
            nc.vector.tensor_copy(warm_out[:], warm_ps[0:1, 0:16])

            ident_b = cpool.tile([128, 128], BF, tag="identb")
            make_identity(nc, ident_b[:])
            ident_f = cpool.tile([128, 128], F32, tag="identf")
            make_identity(nc, ident_f[:])

            # assembled per-batch results
            hidT_sb = cpool.tile([128, KH * BC], F32, tag="hidT")
            wsT_sb = cpool.tile([128, KD * BC], BF, tag="wsT")

            # ---- phase 0: hidden_in = decoder_hidden @ W_b.T ----------------
            # (emitted mid-way through batch 0's big matmuls so the PE does
            # not idle on the wbT DMA at startup)
            def hid_phase():
                hid_sb = once_pool.tile([128, 512], F32, tag="hid")
                psum_hid = wsacc_pool.tile([128, 512], F32, tag="wsacc")
                for k in range(KH):
                    for n in range(2):
                        nc.tensor.matmul(
                            psum_hid[32 * n : 32 * n + BC, :],
                            hT_sb[:, k * BC : (k + 1) * BC],
                            wbT_sb[:, k * HP + n * 512 : k * HP + (n + 1) * 512],
                            start=(k == 0),
                            stop=(k == KH - 1),
                            tile_position=(0, 32 * n),
                            skip_group_check=True,
                        )
                nc.vector.tensor_copy(hid_sb[:], psum_hid[:])
                for h in range(KH):
                    n, hh = divmod(h, 4)
                    pt = sc_pool.tile([128, BC], F32, tag="sc")
                    nc.tensor.transpose(
                        pt[:],
                        hid_sb[32 * n : 32 * n + BC, hh * 128 : (hh + 1) * 128],
                        ident_f[32 * n : 32 * n + BC, 32 * n : 32 * n + BC],
                        tile_position=(32 * n, 0),
                    )
                    nc.vector.tensor_copy(
                        hidT_sb[:, h * BC : (h + 1) * BC], pt[:]
                    )

            # ---- main pipeline over batches ---------------------------------
            ctxN_tiles = [None] * BC
            att_tiles = [None] * BC
            ws_psum = wsacc_pool.tile([128, 512], F32, tag="wsacc", name="wsps")

            def stage_scores(b):
                """big matmul + tanh + scores + masked softmax for batch b."""
                if b == 0:
                    ctxT_t = ctxT0_t
                else:
                    ctxT_t = ctxT_pool.tile([128, KD, S], F8, tag="ctxT")
                    nc.sync.dma_start(ctxT_t[:], ctxT[b])
                if b == 1:
                    # tail-only data; issued here so it doesn't delay startup
                    nc.sync.dma_start(dwT_sb[:], dwT[:])
                    nc.sync.dma_start(dbias_sb[:], dbias[:])
                ctxN_t = ctxN_pool.tile([128, KS * DP], BF, tag="ctxN")
                nc.sync.dma_start(ctxN_t[:], ctxN[b])
                ctxN_tiles[b] = ctxN_t

                psum_sc = sc_pool.tile([1, S], F32, tag="sc")
                tanh_tiles = {}
                ci_tiles = {}

                def emit_ci(h):
                    psum_ci = ci_pool.tile([128, S], F32, tag="ci")
                    for g in range(KD // 2):
                        nc.tensor.matmul(
                            psum_ci[:],
                            waT_sb[:, 2 * g : 2 * g + 2, h * 128 : (h + 1) * 128],
                            ctxT_t[:, 2 * g : 2 * g + 2, :],
                            start=(g == 0),
                            stop=(g == KD // 2 - 1),
                            perf_mode=mybir.MatmulPerfMode.DoubleRow,
                        )
                    ci_tiles[h] = psum_ci

                def emit_tanh(h):
                    tanh_t = tanh_pool.tile([128, S], BF, tag="tanh")
                    nc.scalar.activation(
                        tanh_t[:],
                        ci_tiles.pop(h)[:],
                        mybir.ActivationFunctionType.Tanh,
                        bias=hidT_sb[:, h * BC + b : h * BC + b + 1],
                        scale=1.0 / WA_SCALE,
                    )
                    tanh_tiles[h] = tanh_t

                def emit_scores(h):
                    nc.tensor.matmul(
                        psum_sc[:],
                        wcT_sb[:, h : h + 1],
                        tanh_tiles.pop(h)[:],
                        start=(h == 0),
                        stop=(h == KH - 1),
                    )

                # pipeline: tanh lags ci by `lag` h-blocks so the PE never
                # waits on ACT.  All scores matmuls are emitted after the ci
                # blocks so the big-matmul LDWEIGHTS pipeline is broken once
                # per batch, not once per h-block.
                lag = 2 if b == 0 else 1
                for h in range(KH):
                    emit_ci(h)
                    if b == 0 and h == 2:
                        hid_phase()
                    if h >= lag:
                        emit_tanh(h - lag)
                for h in range(KH - lag, KH):
                    emit_tanh(h)
                for h in range(KH):
                    emit_scores(h)

                # masked softmax on a single partition (512 elements)
                sc_sb = small_pool.tile([1, S], F32, tag="scsb")
                nc.vector.tensor_tensor(
                    sc_sb[:], psum_sc[:], maskv_sb[0:1, b * S : (b + 1) * S], op=mybir.AluOpType.add
                )
                # no max-subtraction: scores are O(1) (exp <= e^4) and
                # masked entries are -1e6 (exp underflows to exactly 0)
                exp_sb = small_pool.tile([1, S], F32, tag="exp")
                esum = small_pool.tile([1, 1], F32, tag="esum")
                nc.scalar.activation(
                    exp_sb[:], sc_sb[:], mybir.ActivationFunctionType.Exp,
                    bias=0.0, scale=1.0, accum_out=esum[:],
                )
                inv = small_pool.tile([1, 1], F32, tag="inv")
                nc.vector.reciprocal(inv[:], esum[:])
                att_sb = small_pool.tile([1, S], BF, tag="att")
                nc.vector.tensor_scalar_mul(att_sb[:], exp_sb[:], inv[:])
                att_tiles[b] = att_sb

            def stage_ws(b):
                """att transpose + weighted sum for batch b (accumulates into
                the persistent ws psum tiles; rows j != b add exactly zero
                because attT_b is zero outside column b)."""
                att_sb = att_tiles[b]
                attT_b = small_pool.tile([128, KS * BC], BF, tag="attTb")
                nc.gpsimd.memset(attT_b[:], 0.0)
                for st in range(KS):
                    pt = sc_pool.tile([128, 1], BF, tag="sc")
                    nc.tensor.transpose(
                        pt[:], att_sb[0:1, st * 128 : (st + 1) * 128],
                        ident_b[0:1, 0:1],
                    )
                    nc.vector.tensor_copy(
                        attT_b[:, st * BC + b : st * BC + b + 1], pt[:]
                    )
                ctxN_t = ctxN_tiles[b]
                for st in range(KS):
                    for nch in range(KS):
                        nc.tensor.matmul(
                            ws_psum[32 * nch : 32 * nch + BC, :],
                            attT_b[:, st * BC : (st + 1) * BC],
                            ctxN_t[:, st * DP + nch * 512 : st * DP + (nch + 1) * 512],
                            start=(b == 0 and st == 0),
                            stop=(b == BC - 1 and st == KS - 1),
                            tile_position=(0, 32 * nch),
                            skip_group_check=True,
                        )

            # software pipeline: scores(b) runs while ws(b-1) consumes
            for b in range(BC + 1):
                if b < BC:
                    stage_scores(b)
                if b >= 1:
                    stage_ws(b - 1)

            # ---- tail: dense layer ------------------------------------------
            ws_col = once_pool.tile([128, 512], BF, tag="wscol")
            nc.vector.tensor_copy(ws_col[:], ws_psum[:])
            for k in range(KD):
                nch, kk = divmod(k, KS)
                pt = sc_pool.tile([128, BC], BF, tag="sc")
                nc.tensor.transpose(
                    pt[:],
                    ws_col[32 * nch : 32 * nch + BC, kk * 128 : (kk + 1) * 128],
                    ident_b[32 * nch : 32 * nch + BC, 32 * nch : 32 * nch + BC],
                    tile_position=(32 * nch, 0),
                )
                nc.vector.tensor_copy(wsT_sb[:, k * BC : (k + 1) * BC], pt[:])

            psum_d = wsacc_pool.tile([128, 512], F32, tag="wsacc")
            for k in range(KD):
                for n in range(2):
                    nc.tensor.matmul(
                        psum_d[32 * n : 32 * n + BC, :],
                        wsT_sb[:, k * BC : (k + 1) * BC],
                        dwT_sb[:, k * HP + n * 512 : k * HP + (n + 1) * 512],
                        start=(k == 0),
                        stop=(k == KD - 1),
                        tile_position=(0, 32 * n),
                        skip_group_check=True,
                    )
            out_sb = once_pool.tile([128, 512], F32, tag="outsb")
            nc.vector.tensor_tensor(
                out_sb[:], psum_d[:], dbias_sb[:], op=mybir.AluOpType.add
            )
            for n in range(2):
                nc.sync.dma_start(out_ext[n], out_sb[32 * n : 32 * n + BC, :])

    nc.compile()
    return nc


_GRAPH = None


def _prep_inputs(decoder_hidden, context, mask, W_a, W_b, W_c_w, W_c_b,
                 dense_w, dense_b):
    """Shard + pad + cast + pack all inputs into per-core input maps."""
    # weights (replicated, packed partition-major over the contraction dim)
    wa = np.zeros((DP, HP), dtype=FP8)
    wa[:D, :H] = (W_a.T.astype(np.float32) * WA_SCALE).astype(FP8)
    waT_p = np.ascontiguousarray(wa.reshape(KD, 128, HP).transpose(1, 0, 2))
    wb = np.zeros((HP, HP), dtype=BF16)
    wb[:H, :H] = W_b.T.astype(BF16)
    wbT_p = _pack_ktiles(wb)
    dw = np.zeros((DP, HP), dtype=BF16)
    dw[:D, :H] = dense_w.T.astype(BF16)
    dwT_p = _pack_ktiles(dw)
    wc = np.zeros((HP, 1), dtype=BF16)
    wc[:H, 0] = W_c_w[0].astype(BF16)
    wcT_p = _pack_ktiles(wc)
    db = np.zeros((HP,), dtype=np.float32)
    db[:H] = dense_b.astype(np.float32)
    dbias_p = np.zeros((128, 512), dtype=np.float32)
    for n in range(2):
        dbias_p[32 * n : 32 * n + BC, :] = db[n * 512 : (n + 1) * 512]

    hid = np.zeros((HP, B), dtype=BF16)
    hid[:H, :] = decoder_hidden[0].T.astype(BF16)   # (H, B)

    maskf = W_c_b.astype(np.float32)[0] - 1e6 * mask[:, :, 0].astype(np.float32)

    in_maps = []
    for c in range(NCORES):
        b0 = c * BC
        ctxf = np.zeros((BC, S, DP), dtype=np.float32)
        ctxf[:, :, :D] = context[b0 : b0 + BC]
        # d-major fp8 packing: [b, p, k, s] = ctx[b, s, k*128+p]
        ctxT_p = np.ascontiguousarray(
            ctxf.transpose(0, 2, 1).astype(FP8).reshape(BC, KD, 128, S)
            .transpose(0, 2, 1, 3)
        )
        # s-major bf16 packing: [b, p, st*DP+d] = ctx[b, st*128+p, d]
        ctxN_p = np.ascontiguousarray(
            ctxf.astype(BF16).reshape(BC, KS, 128, DP).transpose(0, 2, 1, 3)
            .reshape(BC, 128, KS * DP)
        )
        hT_p = _pack_ktiles(np.ascontiguousarray(hid[:, b0 : b0 + BC]))
        in_maps.append({
            "ctxT": ctxT_p,
            "ctxN": ctxN_p,
            "waT": waT_p,
            "wbT": wbT_p,
            "dwT": dwT_p,
            "hT": hT_p,
            "wcT": wcT_p,
            "maskv": np.ascontiguousarray(maskf[b0 : b0 + BC].reshape(1, BC * S)),
            "dbias": dbias_p,
        })
    return in_maps


def kernel(decoder_hidden, context, mask, W_a, W_b, W_c_w, W_c_b,
           dense_w, dense_b, _trace=False):
    global _GRAPH
    if _GRAPH is None:
        _GRAPH = _build_graph()
    in_maps = _prep_inputs(
        np.asarray(decoder_hidden), np.asarray(context), np.asarray(mask),
        np.asarray(W_a), np.asarray(W_b), np.asarray(W_c_w),
        np.asarray(W_c_b), np.asarray(dense_w), np.asarray(dense_b),
    )
    res = run_bass_kernel_spmd(_GRAPH, in_maps, list(range(NCORES)), trace=_trace)
    out = np.concatenate(
        [np.concatenate([res.results[c]["out"][0], res.results[c]["out"][1]],
                        axis=1)[:, :H]
         for c in range(NCORES)], axis=0
    ).astype(np.float32)
    if _trace:
        kernel.last_exec_time_ns = res.exec_time_ns
    return out.reshape(B, 1, H)
